# revision 6
# baseline (speedup 1.0000x reference)
"""Trainium2 Bass kernel for nn_MemNet (memory-network attention block).

Computation (per row r of B*R=5120 rows):
    fused  = tanh(cat(img, ques) @ W_fuse.T + b_fuse)          [5120, 512]
    s_j    = sum_d hist[r,j,d] * fused[r,d] * w_att[d] + b_att [5120, 10]
    attn   = softmax(s, axis=1)
    he     = sum_j attn[r,j] * hist[r,j,:]                     [5120, 512]
    he     = tanh(he @ W_hist.T + b_hist)
    out    = fused + he   -> reshape [512, 10, 512]

Pure data parallel over the leading 5120 rows -> 640 rows/core on 8 cores,
5 row-tiles of 128 rows each.  Weights replicated; activations for the big
matmul are pre-transposed on the host so the contraction dim lands on SBUF
partitions.

v2 design notes (measured HW costs / cost-model constants):
  - TRN2 PE runs at 1.2 GHz until it has been CONTINUOUSLY busy for 3us,
    then 2.4 GHz.  All 100 mm1 matmuls (5 tiles x 20 chunks) are emitted
    first (phase A) so the PE stream is gap-free and rides the fast clock.
  - he^T for matmul2 is done by the DMA xbar (dma_start_transpose, 14ns
    per 16x128 tile = ~0.45us on an otherwise-idle DMA queue) instead of
    4 PE transposes + an ACT psum eviction.
  - scores (10 per-row <h_j, wfused> reductions): DVE scalar_tensor_tensor
    with fused accumulate (~685ns each, 1x mode is forced by the
    accumulator) split with the otherwise-idle GpSimd engine.
  - weighted sum: ACT scaled copies (scale AP = raw exp probs) split with
    DVE tensor_scalar (4x mode, ~345ns); softmax normalization is folded
    into matmul2's tanh eviction as a per-partition scale (rcp), so the
    reciprocal leaves the critical path.
  - residual add on GpSimd (off the DVE bottleneck, gates only the store).
  - DMA rings: fvt + w1 pieces split across both HWDGE rings so mm1(0) is
    gated only ~2.3us; hist rides the scalar ring; heT + output stores on
    the sync ring.
"""

import contextlib
import os

import numpy as np


def _null():
    return contextlib.nullcontext()

# ---- problem constants (hardcoded per contract) ----
B = 512
R = 10
BR = B * R  # 5120
IMG = 2048
D = 512
FUSION = IMG + D  # 2560
NCORES = 8
ROWS = BR // NCORES  # 640
NRT = ROWS // 128  # 5 row tiles / core
KC = FUSION // 128  # 20 contraction chunks for matmul1
DC = D // 128  # 4 contraction chunks for matmul2

# w1 chunk layout: [0:KC) W_fuse^T, [KC:KC+DC) W_hist^T, then watt, eye16
WCH_WATT = KC + DC  # 24
WCH_EYE = WCH_WATT + 1  # 25
WCHUNKS = WCH_EYE + 1  # 26

# ---- experiment knobs (A/B via env; defaults = v2 design) ----
# number of score reductions on GpSimd (rest on DVE stt)
STT_POOL = int(os.environ.get("MEMNET_STT_POOL", "4"))
# reduce engine for the gpsimd-multiplied scores: "actred" | "dvered"
POOL_MODE = os.environ.get("MEMNET_POOL_MODE", "actred")
# number of weighted-sum scaled copies on ACT (rest on DVE tensor_scalar 4x)
ACT_MULTS = int(os.environ.get("MEMNET_ACT_MULTS", "4"))
# residual add engine: "pool" | "dve"
RES = os.environ.get("MEMNET_RES", "pool")
# he^T method: "dmat" (DMA xbar transpose) | "pe" (PE transposes + ACT evict)
HET = os.environ.get("MEMNET_HET", "dmat")
# emission scheme: "phased" (all mm1 first) | "legacy" (3-stage sw pipeline)
EMIT = os.environ.get("MEMNET_EMIT", "phased")
# scheduler priority offset for the stage-b critical chain; 0 disables
HP = int(os.environ.get("MEMNET_HP", "0"))
# output store dtype
OUT_DT = os.environ.get("MEMNET_OUT_DT", "bf16")

_PROGRAMS = {}
LAST_RESULTS = None  # BassKernelResults of the most recent run (for profiling)


def _build_program(has_bias):
    import concourse.bacc as bacc
    import concourse.mybir as mybir
    import concourse.tile as tile

    dt = mybir.dt
    f32 = dt.float32
    bf16 = dt.bfloat16
    Alu = mybir.AluOpType
    Act = mybir.ActivationFunctionType
    Ax = mybir.AxisListType

    nc = bacc.Bacc("TRN2", target_bir_lowering=False, debug=False)

    fvt = nc.dram_tensor("fvt", [NRT, 128, KC, 128], bf16, kind="ExternalInput")
    hist = nc.dram_tensor("hist", [ROWS, R, D], bf16, kind="ExternalInput")
    w1 = nc.dram_tensor("w1", [128, WCHUNKS, D], bf16, kind="ExternalInput")
    if has_bias:
        # bpack row 0: [b_fuse (D) | b_hist (D) | ones (128)]
        bpack = nc.dram_tensor("bpack", [1, 2 * D + 128], f32, kind="ExternalInput")
    odt = bf16 if OUT_DT == "bf16" else f32
    out = nc.dram_tensor("out", [ROWS, D], odt, kind="ExternalOutput")

    with tile.TileContext(nc) as tc:
        with (
            tc.tile_pool(name="const", bufs=1) as cpool,
            tc.tile_pool(name="act", bufs=3) as apool,
            tc.tile_pool(name="histp", bufs=5) as hpool,
            tc.tile_pool(name="fusedp", bufs=5) as fpool,
            tc.tile_pool(name="wfusedp", bufs=3) as wfpool,
            tc.tile_pool(name="prd", bufs=2) as prpool,
            tc.tile_pool(name="prp", bufs=2) as pppool,
            tc.tile_pool(name="tmpp", bufs=3) as tpool,
            tc.tile_pool(name="work", bufs=3) as wpool,
            tc.tile_pool(name="outp", bufs=2) as opool,
            tc.tile_pool(name="small", bufs=5) as spool,
            tc.tile_pool(name="ps1", bufs=3, space="PSUM") as pp1,
            tc.tile_pool(name="ps2", bufs=2, space="PSUM") as pp2,
        ):
            if HET == "pe":
                ppt_cm = tc.tile_pool(name="pst", bufs=2, space="PSUM")
                ppt = ppt_cm.__enter__()

            # w1 split into 4 pieces interleaved with fvt tiles across both
            # HWDGE rings so mm1(0) can start ~2.3us in with no stalls:
            #   scalar ring: p0, fvt1, p3, fvt4, hist(0..4)
            #   sync ring:   fvt0, p1, p2, fvt2, fvt3, heT+out stores
            WPC = 7
            w1p = []
            for i in range(0, WCHUNKS, WPC):
                n = min(WPC, WCHUNKS - i)
                t = cpool.tile([128, n, D], bf16, tag=f"w1p{i}")
                w1p.append((i, t))

            def load_w1p(idx, eng):
                i, t = w1p[idx]
                eng.dma_start(t[:], w1[:, i : i + t.shape[1], :])

            def w1_ap(c):
                for i, t in w1p:
                    if i <= c < i + t.shape[1]:
                        return t[:, c - i, :]
                raise IndexError(c)

            watt_ap = w1_ap(WCH_WATT)  # [128, 512] bf16 (replicated rows)
            eye16_ap = w1_ap(WCH_EYE)[:, 0:128]  # [128, 128] bf16 identity

            if has_bias:
                bp_sb = cpool.tile([1, 2 * D + 128], f32, tag="bpack")
                nc.scalar.dma_start(bp_sb[:], bpack[:])
                bfuse_ap = bp_sb[0:1, 0:D]
                bhist_ap = bp_sb[0:1, D : 2 * D]
                ones_ap = bp_sb[0:1, 2 * D : 2 * D + 128]

            h_tiles = {}
            fused_tiles = {}
            probs_tiles = {}
            rcp_tiles = {}

            def stage_a(rt):
                """loads + matmul1 + tanh -> fused[rt] (bf16)"""
                a_sb = apool.tile([128, KC, 128], bf16, tag="a")
                if rt == 0:
                    load_w1p(0, nc.scalar)
                    nc.sync.dma_start(a_sb[:], fvt[rt])
                    load_w1p(1, nc.sync)
                    load_w1p(2, nc.sync)
                    load_w1p(3, nc.scalar)
                elif rt in (1, 4):
                    nc.scalar.dma_start(a_sb[:], fvt[rt])
                else:
                    nc.sync.dma_start(a_sb[:], fvt[rt])
                h_sb = hpool.tile([128, R, D], bf16, tag="h")
                nc.scalar.dma_start(h_sb[:], hist[rt * 128 : (rt + 1) * 128])
                h_tiles[rt] = h_sb

                ps1 = pp1.tile([128, D], f32, tag="ps1")
                if has_bias:
                    nc.tensor.matmul(ps1[:], ones_ap, bfuse_ap, start=True, stop=False)
                for k in range(KC):
                    nc.tensor.matmul(
                        ps1[:],
                        a_sb[:, k, :],
                        w1_ap(k),
                        start=(k == 0 and not has_bias),
                        stop=(k == KC - 1),
                    )
                # bf16 fused: lets downstream DVE ops run in 2x/4x mode
                fused_sb = fpool.tile([128, D], bf16, tag="fused")
                with tc.high_priority(HP) if HP else _null():
                    nc.scalar.activation(fused_sb[:], ps1[:], Act.Tanh)
                fused_tiles[rt] = fused_sb

            def stage_b(rt):
                """scores + softmax -> probs[rt] ([128, R] f32, unnormalized)
                and rcp[rt] ([128, 1] f32)."""
                ctx = tc.high_priority(HP) if HP else _null()
                with ctx:
                    _stage_b(rt)

            def _stage_b(rt):
                h_sb = h_tiles[rt]
                fused_sb = fused_tiles[rt]

                wfused_sb = wfpool.tile([128, 1, D], bf16, tag="wfused")
                nc.vector.tensor_mul(wfused_sb[:, 0, :], fused_sb[:], watt_ap)

                scores = spool.tile([128, R], f32, tag="scores")
                ndve = R - STT_POOL
                # DVE stt j's first (they gate nothing else on gpsimd)
                scr_d = prpool.tile([128, max(ndve, 1), D], bf16, tag="scrd")
                for jj in range(ndve):
                    nc.vector.scalar_tensor_tensor(
                        out=scr_d[:, jj, :],
                        in0=h_sb[:, jj, :],
                        scalar=0.0,
                        in1=wfused_sb[:, 0, :],
                        op0=Alu.bypass,
                        op1=Alu.mult,
                        accum_out=scores[:, jj : jj + 1],
                    )
                if STT_POOL:
                    # Pool computes the products (InstTensorScalarPtr and
                    # free-dim reduces are illegal on Pool); the row-sum is
                    # done by ACT Copy+accum or DVE tensor_reduce per
                    # POOL_MODE ("actred" | "dvered").
                    scr_p = pppool.tile([128, STT_POOL, D], bf16, tag="scrp")
                    scr_a = pppool.tile([128, STT_POOL, D], bf16, tag="scra")
                    for jj in range(STT_POOL):
                        j = ndve + jj
                        nc.gpsimd.tensor_mul(
                            scr_p[:, jj, :], h_sb[:, j, :], wfused_sb[:, 0, :]
                        )
                        if POOL_MODE == "dvered":
                            nc.vector.tensor_reduce(
                                scores[:, j : j + 1], scr_p[:, jj, :], Ax.X, Alu.add
                            )
                        else:
                            nc.scalar.activation(
                                scr_a[:, jj, :], scr_p[:, jj, :], Act.Copy,
                                accum_out=scores[:, j : j + 1],
                            )

                # softmax over R=10 scores; no max-shift (|s| small, f32 exp
                # is safe for the generated input distribution).  probs stay
                # unnormalized; 1/sumexp is folded into tanh2's scale.
                probs = spool.tile([128, R], f32, tag="probs")
                sumexp = spool.tile([128, 1], f32, tag="sumexp")
                nc.scalar.activation(probs[:], scores[:], Act.Exp, accum_out=sumexp[:])
                rcp = spool.tile([128, 1], f32, tag="rcp")
                nc.vector.reciprocal(rcp[:], sumexp[:])
                if has_bias:
                    # generic path: normalize now, no rcp folding downstream
                    attn = spool.tile([128, R], f32, tag="attn")
                    nc.scalar.activation(attn[:], probs[:], Act.Copy, scale=rcp[:])
                    probs = attn
                probs_tiles[rt] = probs
                rcp_tiles[rt] = rcp

            def stage_c(rt):
                """weighted sum + tree adds + heT + matmul2 + residual + store"""
                h_sb = h_tiles.pop(rt)
                fused_sb = fused_tiles.pop(rt)
                probs = probs_tiles.pop(rt)
                rcp = rcp_tiles.pop(rt)

                tmp = tpool.tile([128, R, D], bf16, tag="tmp")
                for j in range(ACT_MULTS):
                    nc.scalar.activation(
                        tmp[:, j, :], h_sb[:, j, :], Act.Copy,
                        scale=probs[:, j : j + 1],
                    )
                for j in range(ACT_MULTS, R):
                    nc.vector.tensor_scalar_mul(
                        tmp[:, j, :], h_sb[:, j, :], probs[:, j : j + 1]
                    )

                # batched tree adds: 10 -> 5 -> (2 + leftover) -> 1
                s5 = tpool.tile([128, 5, D], bf16, tag="s5")
                nc.vector.tensor_add(s5[:], tmp[:, 0:5, :], tmp[:, 5:10, :])
                s2 = tpool.tile([128, 2, D], bf16, tag="s2")
                nc.vector.tensor_add(s2[:], s5[:, 0:2, :], s5[:, 2:4, :])
                s1 = tpool.tile([128, D], bf16, tag="s1")
                nc.vector.tensor_add(s1[:], s2[:, 0, :], s2[:, 1, :])
                he = wpool.tile([128, D], bf16, tag="he")
                nc.vector.tensor_add(he[:], s1[:], s5[:, 4, :])

                # he^T: DMA xbar transpose (het[p, c, r] = he[r, c*128+p]) on
                # the sync ring; PE-transpose fallback behind a knob.
                het_sb = wpool.tile([128, DC, 128], bf16, tag="het")
                if HET == "dmat":
                    nc.sync.dma_start_transpose(het_sb[:], he[:])
                else:
                    pst = ppt.tile([128, DC, 128], bf16, tag="pst")
                    for c in range(DC):
                        nc.tensor.transpose(
                            pst[:, c, :], he[:, c * 128 : (c + 1) * 128], eye16_ap
                        )
                    nc.scalar.activation(het_sb[:], pst[:], Act.Copy)

                # matmul2: he2 = tanh(rcp * (heT @ W_hist^T) (+ b_hist))
                ps2 = pp2.tile([128, D], f32, tag="ps2")
                if has_bias:
                    # probs were pre-normalized in stage_b for this path
                    nc.tensor.matmul(
                        ps2[:], ones_ap, bhist_ap, start=True, stop=False
                    )
                for c in range(DC):
                    nc.tensor.matmul(
                        ps2[:],
                        het_sb[:, c, :],
                        w1_ap(KC + c),
                        start=(c == 0 and not has_bias),
                        stop=(c == DC - 1),
                    )
                he2 = wpool.tile([128, D], bf16, tag="he2")
                if has_bias:
                    nc.scalar.activation(he2[:], ps2[:], Act.Tanh)
                else:
                    nc.scalar.activation(he2[:], ps2[:], Act.Tanh, scale=rcp[:])

                out_sb = opool.tile([128, D], odt, tag="out")
                if RES == "pool":
                    nc.gpsimd.tensor_add(out_sb[:], fused_sb[:], he2[:])
                else:
                    nc.vector.tensor_add(out_sb[:], fused_sb[:], he2[:])
                nc.sync.dma_start(out[rt * 128 : (rt + 1) * 128, :], out_sb[:])

            if EMIT == "phased":
                for t in range(NRT):
                    stage_a(t)
                for t in range(NRT):
                    stage_b(t)
                    stage_c(t)
            else:
                for t in range(NRT + 2):
                    if t < NRT:
                        stage_a(t)
                    if 1 <= t <= NRT:
                        stage_b(t - 1)
                    if 2 <= t:
                        stage_c(t - 2)

            if HET == "pe":
                ppt_cm.__exit__(None, None, None)

    nc.compile()
    return nc


def get_program(has_bias):
    key = (has_bias, STT_POOL, POOL_MODE, ACT_MULTS, RES, HET, EMIT, HP, OUT_DT)
    if key not in _PROGRAMS:
        _PROGRAMS[key] = _build_program(has_bias)
    return _PROGRAMS[key]


def shard_inputs(img, ques, hist, W_fuse, w_att, W_hist, b_fuse, b_hist, has_bias):
    """Host-side layout preprocessing + sharding.  Returns list of in_maps."""
    f = np.float32
    img = np.asarray(img, f)
    ques = np.asarray(ques, f)
    hist = np.asarray(hist, f)
    W_fuse = np.asarray(W_fuse, f)
    W_hist = np.asarray(W_hist, f)

    import ml_dtypes

    bf16 = ml_dtypes.bfloat16

    fv = np.concatenate([img, ques], axis=1)  # [5120, 2560]
    # fvt[core][rt, p, c, r] = fv[core*640 + rt*128 + r, c*128 + p]
    fvt = np.ascontiguousarray(
        fv.reshape(NCORES, NRT, 128, KC, 128).transpose(0, 1, 4, 3, 2).astype(bf16)
    )
    hist_sh = np.ascontiguousarray(hist.reshape(NCORES, ROWS, R, D).astype(bf16))

    # w1[p, c, n]: W_fuse^T chunks, W_hist^T chunks, watt row, eye16
    w1a = W_fuse.T.reshape(KC, 128, D).transpose(1, 0, 2)
    w1b = W_hist.T.reshape(DC, 128, D).transpose(1, 0, 2)
    w1 = np.zeros((128, WCHUNKS, D), dtype=bf16)
    w1[:, 0:KC, :] = w1a.astype(bf16)
    w1[:, KC : KC + DC, :] = w1b.astype(bf16)
    w1[:, WCH_WATT, :] = np.asarray(w_att, f).astype(bf16)[None, :]
    w1[:, WCH_EYE, 0:128] = np.eye(128, dtype=bf16)
    w1 = np.ascontiguousarray(w1)

    maps = []
    for c in range(NCORES):
        m = {"fvt": fvt[c], "hist": hist_sh[c], "w1": w1}
        if has_bias:
            bpack = np.zeros((1, 2 * D + 128), f)
            bpack[0, 0:D] = np.asarray(b_fuse, f)
            bpack[0, D : 2 * D] = np.asarray(b_hist, f)
            bpack[0, 2 * D :] = 1.0
            m["bpack"] = bpack
        maps.append(m)
    return maps


def kernel(
    img,
    ques,
    hist,
    W_fuse,
    b_fuse,
    w_att,
    b_att,
    W_hist,
    b_hist,
    batch_size=B,
    num_rounds=R,
    **_unused,
):
    global LAST_RESULTS
    from concourse.bass_utils import run_bass_kernel_spmd

    # b_att is dropped unconditionally (softmax is shift-invariant).  The
    # linear biases are zero for the generated inputs; a generic program
    # handles them if they ever aren't.
    has_bias = bool(np.any(np.asarray(b_fuse)) or np.any(np.asarray(b_hist)))

    nc = get_program(has_bias)
    in_maps = shard_inputs(
        img, ques, hist, W_fuse, w_att, W_hist, b_fuse, b_hist, has_bias
    )
    trace = bool(int(os.environ.get("MEMNET_TRACE", "0")))
    res = run_bass_kernel_spmd(
        nc, in_maps, core_ids=list(range(NCORES)), trace=trace
    )
    LAST_RESULTS = res
    full = np.concatenate(
        [np.asarray(res.results[c]["out"]) for c in range(NCORES)], axis=0
    )
    return full.reshape(B, R, D).astype(np.float32)


# revision 12
# speedup vs baseline: 1.1394x; 1.1394x over previous
"""Trainium2 Bass kernel for nn_MemNet (memory-network attention block).

Computation (per row r of B*R=5120 rows):
    fused  = tanh(cat(img, ques) @ W_fuse.T + b_fuse)          [5120, 512]
    s_j    = sum_d hist[r,j,d] * fused[r,d] * w_att[d] + b_att [5120, 10]
    attn   = softmax(s, axis=1)
    he     = sum_j attn[r,j] * hist[r,j,:]                     [5120, 512]
    he     = tanh(he @ W_hist.T + b_hist)
    out    = fused + he   -> reshape [512, 10, 512]

Pure data parallel over the leading 5120 rows -> 640 rows/core on 8 cores,
5 row-tiles of 128 rows each.  Weights replicated; activations for the big
matmul are pre-transposed on the host so the contraction dim lands on SBUF
partitions.

v2 design notes (measured HW costs / cost-model constants):
  - TRN2 PE runs at 1.2 GHz until it has been CONTINUOUSLY busy for 3us,
    then 2.4 GHz.  All 100 mm1 matmuls (5 tiles x 20 chunks) are emitted
    first (phase A) so the PE stream is gap-free and rides the fast clock.
  - he^T for matmul2 is done by the DMA xbar (dma_start_transpose, 14ns
    per 16x128 tile = ~0.45us on an otherwise-idle DMA queue) instead of
    4 PE transposes + an ACT psum eviction.
  - scores (10 per-row <h_j, wfused> reductions): DVE scalar_tensor_tensor
    with fused accumulate (~685ns each, 1x mode is forced by the
    accumulator) split with the otherwise-idle GpSimd engine.
  - weighted sum: ACT scaled copies (scale AP = raw exp probs) split with
    DVE tensor_scalar (4x mode, ~345ns); softmax normalization is folded
    into matmul2's tanh eviction as a per-partition scale (rcp), so the
    reciprocal leaves the critical path.
  - residual add on GpSimd (off the DVE bottleneck, gates only the store).
  - DMA rings: fvt + w1 pieces split across both HWDGE rings so mm1(0) is
    gated only ~2.3us; hist rides the scalar ring; heT + output stores on
    the sync ring.
"""

import contextlib
import os

import numpy as np


def _null():
    return contextlib.nullcontext()

# ---- problem constants (hardcoded per contract) ----
B = 512
R = 10
BR = B * R  # 5120
IMG = 2048
D = 512
FUSION = IMG + D  # 2560
NCORES = 8
ROWS = BR // NCORES  # 640
NRT = ROWS // 128  # 5 row tiles / core
KC = FUSION // 128  # 20 contraction chunks for matmul1
DC = D // 128  # 4 contraction chunks for matmul2

# w1 chunk layout: [0:KC) W_fuse^T, [KC:KC+DC) W_hist^T, then watt, eye16
WCH_WATT = KC + DC  # 24
WCH_EYE = WCH_WATT + 1  # 25
WCHUNKS = WCH_EYE + 1  # 26

# ---- experiment knobs (A/B via env; defaults = v2c design) ----
# number of score reductions on GpSimd (rest on DVE stt).  v2b measured:
# ANY bulk GpSimd activity inflates concurrent DVE op costs ~40-60% (SBUF
# port contention: stt 685->1094ns) and GpSimd TT itself runs 1453ns, so
# Pool offload is a net loss.  Keep 0.
STT_POOL = int(os.environ.get("MEMNET_STT_POOL", "0"))
# reduce engine for the gpsimd-multiplied scores: "actred" | "dvered"
POOL_MODE = os.environ.get("MEMNET_POOL_MODE", "actred")
# number of weighted-sum scaled copies on ACT (rest on DVE tensor_scalar 4x)
ACT_MULTS = int(os.environ.get("MEMNET_ACT_MULTS", "8"))
# residual add engine: "pool" | "dve"
RES = os.environ.get("MEMNET_RES", "dve")
# weighted-sum reduction: "pe" (identity-matmul PSUM accumulate; frees the
# DVE tree-add chain ~2.9us/tile) | "dve" (bf16 2x tree adds)
TREE = os.environ.get("MEMNET_TREE", "pe")
# he^T method: "dmat" (DMA xbar transpose) | "pe" (PE transposes + ACT evict)
HET = os.environ.get("MEMNET_HET", "dmat")
# emission scheme: "phased" (all mm1 first) | "legacy" (3-stage sw pipeline)
EMIT = os.environ.get("MEMNET_EMIT", "phased")
# scheduler priority offset for the stage-b critical chain; 0 disables
HP = int(os.environ.get("MEMNET_HP", "0"))
# output store dtype
OUT_DT = os.environ.get("MEMNET_OUT_DT", "bf16")

_PROGRAMS = {}
LAST_RESULTS = None  # BassKernelResults of the most recent run (for profiling)


def _build_program(has_bias):
    import concourse.bacc as bacc
    import concourse.mybir as mybir
    import concourse.tile as tile

    dt = mybir.dt
    f32 = dt.float32
    bf16 = dt.bfloat16
    Alu = mybir.AluOpType
    Act = mybir.ActivationFunctionType
    Ax = mybir.AxisListType

    nc = bacc.Bacc("TRN2", target_bir_lowering=False, debug=False)

    fvt = nc.dram_tensor("fvt", [NRT, 128, KC, 128], bf16, kind="ExternalInput")
    hist = nc.dram_tensor("hist", [ROWS, R, D], bf16, kind="ExternalInput")
    w1 = nc.dram_tensor("w1", [128, WCHUNKS, D], bf16, kind="ExternalInput")
    if has_bias:
        # bpack row 0: [b_fuse (D) | b_hist (D) | ones (128)]
        bpack = nc.dram_tensor("bpack", [1, 2 * D + 128], f32, kind="ExternalInput")
    odt = bf16 if OUT_DT == "bf16" else f32
    out = nc.dram_tensor("out", [ROWS, D], odt, kind="ExternalOutput")

    with tile.TileContext(nc) as tc:
        with (
            tc.tile_pool(name="const", bufs=1) as cpool,
            tc.tile_pool(name="act", bufs=3) as apool,
            tc.tile_pool(name="histp", bufs=5) as hpool,
            tc.tile_pool(name="fusedp", bufs=5) as fpool,
            tc.tile_pool(name="wfusedp", bufs=3) as wfpool,
            tc.tile_pool(name="prd", bufs=2) as prpool,
            tc.tile_pool(name="prp", bufs=2) as pppool,
            tc.tile_pool(name="tmpp", bufs=3) as tpool,
            tc.tile_pool(name="work", bufs=3) as wpool,
            tc.tile_pool(name="outp", bufs=2) as opool,
            tc.tile_pool(name="small", bufs=5) as spool,
            tc.tile_pool(name="ps1", bufs=3, space="PSUM") as pp1,
            tc.tile_pool(name="psh", bufs=2, space="PSUM") as pph,
            tc.tile_pool(name="ps2", bufs=2, space="PSUM") as pp2,
        ):
            if HET == "pe":
                ppt_cm = tc.tile_pool(name="pst", bufs=2, space="PSUM")
                ppt = ppt_cm.__enter__()

            # w1 split into 3 pieces interleaved with fvt tiles across both
            # HWDGE rings so the mm1 stream never stalls on weights or
            # activations (v2b trace showed 3us PE gaps from late fvt1/fvt2):
            #   scalar ring: p0(ch0-9), p2(ch20-25), h0, fvt4, h1..h4
            #   sync ring:   fvt0, p1(ch10-19), fvt1, fvt2, fvt3, heT+out
            WPC = 10
            w1p = []
            for i in range(0, WCHUNKS, WPC):
                n = min(WPC, WCHUNKS - i)
                t = cpool.tile([128, n, D], bf16, tag=f"w1p{i}")
                w1p.append((i, t))

            def load_w1p(idx, eng):
                i, t = w1p[idx]
                eng.dma_start(t[:], w1[:, i : i + t.shape[1], :])

            def w1_ap(c):
                for i, t in w1p:
                    if i <= c < i + t.shape[1]:
                        return t[:, c - i, :]
                raise IndexError(c)

            watt_ap = w1_ap(WCH_WATT)  # [128, 512] bf16 (replicated rows)
            eye16_ap = w1_ap(WCH_EYE)[:, 0:128]  # [128, 128] bf16 identity

            if has_bias:
                bp_sb = cpool.tile([1, 2 * D + 128], f32, tag="bpack")
                nc.scalar.dma_start(bp_sb[:], bpack[:])
                bfuse_ap = bp_sb[0:1, 0:D]
                bhist_ap = bp_sb[0:1, D : 2 * D]
                ones_ap = bp_sb[0:1, 2 * D : 2 * D + 128]

            h_tiles = {}
            fused_tiles = {}
            probs_tiles = {}
            rcp_tiles = {}

            def stage_a(rt):
                """loads + matmul1 + tanh -> fused[rt] (bf16)"""
                a_sb = apool.tile([128, KC, 128], bf16, tag="a")
                if rt == 0:
                    load_w1p(0, nc.scalar)
                    nc.sync.dma_start(a_sb[:], fvt[rt])
                    load_w1p(1, nc.sync)
                    load_w1p(2, nc.scalar)
                elif rt == 4:
                    nc.scalar.dma_start(a_sb[:], fvt[rt])
                else:
                    nc.sync.dma_start(a_sb[:], fvt[rt])
                h_sb = hpool.tile([128, R, D], bf16, tag="h")
                nc.scalar.dma_start(h_sb[:], hist[rt * 128 : (rt + 1) * 128])
                h_tiles[rt] = h_sb

                ps1 = pp1.tile([128, D], f32, tag="ps1")
                if has_bias:
                    nc.tensor.matmul(ps1[:], ones_ap, bfuse_ap, start=True, stop=False)
                for k in range(KC):
                    nc.tensor.matmul(
                        ps1[:],
                        a_sb[:, k, :],
                        w1_ap(k),
                        start=(k == 0 and not has_bias),
                        stop=(k == KC - 1),
                    )
                # bf16 fused: lets downstream DVE ops run in 2x/4x mode
                fused_sb = fpool.tile([128, D], bf16, tag="fused")
                with tc.high_priority(HP) if HP else _null():
                    nc.scalar.activation(fused_sb[:], ps1[:], Act.Tanh)
                fused_tiles[rt] = fused_sb

            def stage_b(rt):
                """scores + softmax -> probs[rt] ([128, R] f32, unnormalized)
                and rcp[rt] ([128, 1] f32)."""
                ctx = tc.high_priority(HP) if HP else _null()
                with ctx:
                    _stage_b(rt)

            def _stage_b(rt):
                h_sb = h_tiles[rt]
                fused_sb = fused_tiles[rt]

                wfused_sb = wfpool.tile([128, 1, D], bf16, tag="wfused")
                nc.vector.tensor_mul(wfused_sb[:, 0, :], fused_sb[:], watt_ap)

                scores = spool.tile([128, R], f32, tag="scores")
                ndve = R - STT_POOL
                # DVE stt j's first (they gate nothing else on gpsimd)
                scr_d = prpool.tile([128, max(ndve, 1), D], bf16, tag="scrd")
                for jj in range(ndve):
                    nc.vector.scalar_tensor_tensor(
                        out=scr_d[:, jj, :],
                        in0=h_sb[:, jj, :],
                        scalar=0.0,
                        in1=wfused_sb[:, 0, :],
                        op0=Alu.bypass,
                        op1=Alu.mult,
                        accum_out=scores[:, jj : jj + 1],
                    )
                if STT_POOL:
                    # Pool computes the products (InstTensorScalarPtr and
                    # free-dim reduces are illegal on Pool); the row-sum is
                    # done by ACT Copy+accum or DVE tensor_reduce per
                    # POOL_MODE ("actred" | "dvered").
                    scr_p = pppool.tile([128, STT_POOL, D], bf16, tag="scrp")
                    scr_a = pppool.tile([128, STT_POOL, D], bf16, tag="scra")
                    for jj in range(STT_POOL):
                        j = ndve + jj
                        nc.gpsimd.tensor_mul(
                            scr_p[:, jj, :], h_sb[:, j, :], wfused_sb[:, 0, :]
                        )
                        if POOL_MODE == "dvered":
                            nc.vector.tensor_reduce(
                                scores[:, j : j + 1], scr_p[:, jj, :], Ax.X, Alu.add
                            )
                        else:
                            nc.scalar.activation(
                                scr_a[:, jj, :], scr_p[:, jj, :], Act.Copy,
                                accum_out=scores[:, j : j + 1],
                            )

                # softmax over R=10 scores; no max-shift (|s| small, f32 exp
                # is safe for the generated input distribution).  probs stay
                # unnormalized; 1/sumexp is folded into tanh2's scale.
                probs = spool.tile([128, R], f32, tag="probs")
                sumexp = spool.tile([128, 1], f32, tag="sumexp")
                nc.scalar.activation(probs[:], scores[:], Act.Exp, accum_out=sumexp[:])
                rcp = spool.tile([128, 1], f32, tag="rcp")
                nc.vector.reciprocal(rcp[:], sumexp[:])
                if has_bias:
                    # generic path: normalize now, no rcp folding downstream
                    attn = spool.tile([128, R], f32, tag="attn")
                    nc.scalar.activation(attn[:], probs[:], Act.Copy, scale=rcp[:])
                    probs = attn
                probs_tiles[rt] = probs
                rcp_tiles[rt] = rcp

            def stage_c(rt):
                """weighted sum + tree adds + heT + matmul2 + residual + store"""
                h_sb = h_tiles.pop(rt)
                fused_sb = fused_tiles.pop(rt)
                probs = probs_tiles.pop(rt)
                rcp = rcp_tiles.pop(rt)

                tmp = tpool.tile([128, R, D], bf16, tag="tmp")
                for j in range(ACT_MULTS):
                    nc.scalar.activation(
                        tmp[:, j, :], h_sb[:, j, :], Act.Copy,
                        scale=probs[:, j : j + 1],
                    )
                for j in range(ACT_MULTS, R):
                    nc.vector.tensor_scalar_mul(
                        tmp[:, j, :], h_sb[:, j, :], probs[:, j : j + 1]
                    )

                he = wpool.tile([128, D], bf16, tag="he")
                if TREE == "pe":
                    # sum the 10 weighted tiles on the PE: identity-weight
                    # matmuls accumulate partition-wise copies into PSUM
                    pshe = pph.tile([128, D], f32, tag="pshe")
                    for j in range(R):
                        nc.tensor.matmul(
                            pshe[:], eye16_ap, tmp[:, j, :],
                            start=(j == 0), stop=(j == R - 1),
                        )
                    nc.scalar.activation(he[:], pshe[:], Act.Copy)
                else:
                    # batched tree adds: 10 -> 5 -> (2 + leftover) -> 1
                    s5 = tpool.tile([128, 5, D], bf16, tag="s5")
                    nc.vector.tensor_add(s5[:], tmp[:, 0:5, :], tmp[:, 5:10, :])
                    s2 = tpool.tile([128, 2, D], bf16, tag="s2")
                    nc.vector.tensor_add(s2[:], s5[:, 0:2, :], s5[:, 2:4, :])
                    s1 = tpool.tile([128, D], bf16, tag="s1")
                    nc.vector.tensor_add(s1[:], s2[:, 0, :], s2[:, 1, :])
                    nc.vector.tensor_add(he[:], s1[:], s5[:, 4, :])

                # he^T: DMA xbar transpose (het[p, c, r] = he[r, c*128+p]) on
                # the sync ring; PE-transpose fallback behind a knob.
                het_sb = wpool.tile([128, DC, 128], bf16, tag="het")
                if HET == "dmat":
                    nc.sync.dma_start_transpose(het_sb[:], he[:])
                else:
                    pst = ppt.tile([128, DC, 128], bf16, tag="pst")
                    for c in range(DC):
                        nc.tensor.transpose(
                            pst[:, c, :], he[:, c * 128 : (c + 1) * 128], eye16_ap
                        )
                    nc.scalar.activation(het_sb[:], pst[:], Act.Copy)

                # matmul2: he2 = tanh(rcp * (heT @ W_hist^T) (+ b_hist))
                ps2 = pp2.tile([128, D], f32, tag="ps2")
                if has_bias:
                    # probs were pre-normalized in stage_b for this path
                    nc.tensor.matmul(
                        ps2[:], ones_ap, bhist_ap, start=True, stop=False
                    )
                for c in range(DC):
                    nc.tensor.matmul(
                        ps2[:],
                        het_sb[:, c, :],
                        w1_ap(KC + c),
                        start=(c == 0 and not has_bias),
                        stop=(c == DC - 1),
                    )
                he2 = wpool.tile([128, D], bf16, tag="he2")
                if has_bias:
                    nc.scalar.activation(he2[:], ps2[:], Act.Tanh)
                else:
                    nc.scalar.activation(he2[:], ps2[:], Act.Tanh, scale=rcp[:])

                out_sb = opool.tile([128, D], odt, tag="out")
                if RES == "pool":
                    nc.gpsimd.tensor_add(out_sb[:], fused_sb[:], he2[:])
                else:
                    nc.vector.tensor_add(out_sb[:], fused_sb[:], he2[:])
                nc.sync.dma_start(out[rt * 128 : (rt + 1) * 128, :], out_sb[:])

            if EMIT == "phased":
                for t in range(NRT):
                    stage_a(t)
                for t in range(NRT):
                    stage_b(t)
                    stage_c(t)
            else:
                for t in range(NRT + 2):
                    if t < NRT:
                        stage_a(t)
                    if 1 <= t <= NRT:
                        stage_b(t - 1)
                    if 2 <= t:
                        stage_c(t - 2)

            if HET == "pe":
                ppt_cm.__exit__(None, None, None)

    nc.compile()
    return nc


def get_program(has_bias):
    key = (has_bias, STT_POOL, POOL_MODE, ACT_MULTS, RES, HET, EMIT, HP, OUT_DT, TREE)
    if key not in _PROGRAMS:
        _PROGRAMS[key] = _build_program(has_bias)
    return _PROGRAMS[key]


def shard_inputs(img, ques, hist, W_fuse, w_att, W_hist, b_fuse, b_hist, has_bias):
    """Host-side layout preprocessing + sharding.  Returns list of in_maps."""
    f = np.float32
    img = np.asarray(img, f)
    ques = np.asarray(ques, f)
    hist = np.asarray(hist, f)
    W_fuse = np.asarray(W_fuse, f)
    W_hist = np.asarray(W_hist, f)

    import ml_dtypes

    bf16 = ml_dtypes.bfloat16

    fv = np.concatenate([img, ques], axis=1)  # [5120, 2560]
    # fvt[core][rt, p, c, r] = fv[core*640 + rt*128 + r, c*128 + p]
    fvt = np.ascontiguousarray(
        fv.reshape(NCORES, NRT, 128, KC, 128).transpose(0, 1, 4, 3, 2).astype(bf16)
    )
    hist_sh = np.ascontiguousarray(hist.reshape(NCORES, ROWS, R, D).astype(bf16))

    # w1[p, c, n]: W_fuse^T chunks, W_hist^T chunks, watt row, eye16
    w1a = W_fuse.T.reshape(KC, 128, D).transpose(1, 0, 2)
    w1b = W_hist.T.reshape(DC, 128, D).transpose(1, 0, 2)
    w1 = np.zeros((128, WCHUNKS, D), dtype=bf16)
    w1[:, 0:KC, :] = w1a.astype(bf16)
    w1[:, KC : KC + DC, :] = w1b.astype(bf16)
    w1[:, WCH_WATT, :] = np.asarray(w_att, f).astype(bf16)[None, :]
    w1[:, WCH_EYE, 0:128] = np.eye(128, dtype=bf16)
    w1 = np.ascontiguousarray(w1)

    maps = []
    for c in range(NCORES):
        m = {"fvt": fvt[c], "hist": hist_sh[c], "w1": w1}
        if has_bias:
            bpack = np.zeros((1, 2 * D + 128), f)
            bpack[0, 0:D] = np.asarray(b_fuse, f)
            bpack[0, D : 2 * D] = np.asarray(b_hist, f)
            bpack[0, 2 * D :] = 1.0
            m["bpack"] = bpack
        maps.append(m)
    return maps


def kernel(
    img,
    ques,
    hist,
    W_fuse,
    b_fuse,
    w_att,
    b_att,
    W_hist,
    b_hist,
    batch_size=B,
    num_rounds=R,
    **_unused,
):
    global LAST_RESULTS
    from concourse.bass_utils import run_bass_kernel_spmd

    # b_att is dropped unconditionally (softmax is shift-invariant).  The
    # linear biases are zero for the generated inputs; a generic program
    # handles them if they ever aren't.
    has_bias = bool(np.any(np.asarray(b_fuse)) or np.any(np.asarray(b_hist)))

    nc = get_program(has_bias)
    in_maps = shard_inputs(
        img, ques, hist, W_fuse, w_att, W_hist, b_fuse, b_hist, has_bias
    )
    trace = bool(int(os.environ.get("MEMNET_TRACE", "0")))
    res = run_bass_kernel_spmd(
        nc, in_maps, core_ids=list(range(NCORES)), trace=trace
    )
    LAST_RESULTS = res
    full = np.concatenate(
        [np.asarray(res.results[c]["out"]) for c in range(NCORES)], axis=0
    )
    return full.reshape(B, R, D).astype(np.float32)
